# revision 5
# baseline (speedup 1.0000x reference)
"""Chamfer distance (symmetric, weighted forward) on 8 Trainium2 NeuronCores.

Strategy
--------
Brute-force all-pairs squared distances on the TensorEngine via the augmented
matmul  ||s||^2 + ||t||^2 - 2 s.t  with every fp32 operand split into 3 bf16
planes (products of bf16 planes are exact in fp32), so the PE computes
fp32-accurate squared distances at full bf16 streaming speed.

Sharding: 2 cores per batch element (B=4), each core takes 4096 of the 8192
source rows x all 8192 targets.  Within a core, source blocks are processed in
PAIRS whose K=32 weight sets sit at PE partition strips 0-31 / 32-63 (row-group
packed matmuls), so PE work never gates the pipeline.

Post-matmul pipeline:
  - ACT evacuates each PSUM tile [128, 2048] to SBUF fp16 with scale=-1
    (1x rate, ~2.36us measured) into a per-block [128, 8192] strip.  ACT and
    DVE are the only engines that can read PSUM (DMA has no PSUM route), so
    the evac rate bounds everything at ~300us/core of ACT time.
  - Backward (per-target): DVE folds each strip into the accumulator
    A [128, 8192] with one FD-8192 fp16 tensor_tensor max (2x_1P, ~5.35us);
    31 folds total -- the minimum.
  - Forward (per-source): ONE custom DVE op per strip (TT_MAX_REDUCE_CHAMFER,
    registered at import via the documented dve_ops.OPS extension point):
    body = max(Src0, Src1) over the strip halves (in-place), with
    accum=MAX writing the full row-max straight to racc[:, b].  This replaces
    the old 6-instruction TT pyramid + batched tensor_reduce (~6.9us/strip)
    with a single ~4.6us 1x instruction, taking DVE off the critical path.
  - Input DMAs are chunked so the first matmuls start ~11us in instead of 16.
Dead ends (hardware-verified): tensor_tensor_reduce and non-contiguous 3D-AP
TTs fault the DVE via this runtime; pool/tensor_reduce/max8 have only 1x uops;
GpSimd rejects TENSOR_TENSOR at codegen and its tensor_reduce is C-axis only.

The host builds the bf16 split planes, runs the SPMD kernel, folds the
[128 x 32-block] partition structure, takes sqrt and means.  fp16 rounding of
squared distances costs ~2.4e-4 relative on the result -- far inside the
2e-2 gate (measured end-to-end rel err 6e-7).
"""

import os
import sys

import numpy as np

for _p in ("/root/.axon_site", "/root/.axon_site/_ro/trn_rl_repo", "/root/.axon_site/_ro/pypackages"):
    if os.path.isdir(_p) and _p not in sys.path:
        sys.path.append(_p)

import ml_dtypes

BF16 = ml_dtypes.bfloat16

# Problem constants (hardcoded per spec)
B = 4
N = 8192  # sources per batch
M = 8192  # targets per batch
NCORES = 8
SRC_PER_CORE = N // 2        # 4096
NBLK = SRC_PER_CORE // 128   # 32 source blocks per core
NSUP = M // 2048             # 4 target supertiles per batch
KROWS = 32
EPS = 1e-8

_PROGRAM = None  # cached (nc, ...) build
_TTMAX = None    # cached custom DVE op


def _get_ttmax_op():
    """Register (once) the fused max+row-max-reduce custom DVE op.

    body: out[k] = max(in0[k], in1[k]); accum_out = max_k out[k]
    (accum seeded with the MAX identity -FLT_MAX).
    Registered via the documented extension point (dve_ops.OPS append); the
    per-NEFF uop table is generated from the module-level OPS list at
    compile time, so this must run before nc.compile().
    """
    global _TTMAX
    if _TTMAX is not None:
        return _TTMAX
    from concourse import dve_ops as DO

    name = "TT_MAX_REDUCE_CHAMFER"
    for op in DO.OPS:
        if op.name == name:
            _TTMAX = op
            return op

    from concourse.dve_spec import Spec, Src0, Src1, lower, maxx
    from concourse.dve_uop import DveOpSpec

    def _ref(in0, in1, s0, s1, imm2):
        b = np.maximum(in0.astype(np.float32), in1.astype(np.float32))
        return b, b.reshape(b.shape[0], -1).max(-1, keepdims=True)

    spec = Spec(body=maxx(Src0, Src1), accum=maxx, reference=_ref)
    # Pin the sha from this very build (no external golden needed).
    shas = {
        ver: DveOpSpec(name=name, opcode=0x1F, uops=lower(spec, ver=ver), rd1_en=True).sha(ver)
        for ver in ("v3", "v4")
    }
    op = DO.DveOp(name, spec, subdim=False, uops_sha=shas)
    row = DO._CUSTOM_DVE_ROW_BASE + len(DO.OPS)
    assert row < 0x20, "custom-DVE opcode row overflow"
    DO.OPS.append(op)
    DO.CUSTOM_DVE_SPECS[name] = spec
    DO._SUB_OPCODE_FOR_NAME[name] = row
    _TTMAX = op
    return op


def _splitn(x, n):
    """Split fp64 array into n bf16 planes summing (to ~8n bits) to x."""
    x = x.astype(np.float64)
    out = []
    for _ in range(n):
        a = x.astype(BF16)
        out.append(a)
        x = x - a.astype(np.float64)
    return out


def _build_planes(src_b, tgt_b):
    """Augmented K=32 bf16 planes for one batch.

    Returns L [32, N] (source side / lhsT) and R [32, M] (target side / rhs)
    such that sum_k L[k, n] * R[k, m] == ||s_n - t_m||^2 up to fp32 rounding.
    """
    sa, sb, sc = _splitn(-2.0 * src_b.astype(np.float64), 3)  # (N, 3) each
    ta, tb, tc = _splitn(tgt_b.astype(np.float64), 3)
    ns = (src_b.astype(np.float64) ** 2).sum(1)
    nt = (tgt_b.astype(np.float64) ** 2).sum(1)
    nss = _splitn(ns, 4)
    nts = _splitn(nt, 4)
    one_s = np.ones(ns.shape, BF16)
    one_t = np.ones(nt.shape, BF16)
    Ls, Rs = [], []
    for k in range(3):
        # products: ad ae af bd be bf cd ce (only c*f dropped, ~2^-32 rel)
        for (u, v) in [(sa, ta), (sa, tb), (sa, tc), (sb, ta), (sb, tb), (sb, tc), (sc, ta), (sc, tb)]:
            Ls.append(u[:, k])
            Rs.append(v[:, k])
    for u in nss:
        Ls.append(u)
        Rs.append(one_t)
    for v in nts:
        Ls.append(one_s)
        Rs.append(v)
    L = np.ascontiguousarray(np.stack(Ls, 0).astype(BF16))
    R = np.ascontiguousarray(np.stack(Rs, 0).astype(BF16))
    return L, R


def _build_program():
    """Build the SPMD Tile program once. Returns the finalized Bass object."""
    import concourse.bacc as bacc
    import concourse.tile as tile
    from concourse import mybir

    ttmax = _get_ttmax_op()

    nc = bacc.Bacc("TRN2", target_bir_lowering=False, debug=False, num_devices=NCORES)

    # lhsT packs PAIRS of 128-source blocks at partition strips 0-31 / 32-63
    # so the two K=32 matmuls run concurrently on distinct PE row-groups.
    lhsT_d = nc.dram_tensor("lhsT", [2 * KROWS, SRC_PER_CORE // 2], mybir.dt.bfloat16, kind="ExternalInput")
    rhs_d = nc.dram_tensor("rhs", [2 * KROWS, M], mybir.dt.bfloat16, kind="ExternalInput")
    s2t_d = nc.dram_tensor("s2t", [128, NBLK], mybir.dt.float32, kind="ExternalOutput")
    t2s_d = nc.dram_tensor("t2s", [128, M], mybir.dt.float16, kind="ExternalOutput")

    FMAX = mybir.AluOpType.max
    F16 = mybir.dt.float16

    with tile.TileContext(nc) as tc:
        with (
            tc.tile_pool(name="weights", bufs=1) as wpool,
            tc.tile_pool(name="psum", bufs=2, space="PSUM") as pspool,
            tc.tile_pool(name="evac", bufs=12) as epool,
        ):
            lhsT_sb = wpool.tile([2 * KROWS, SRC_PER_CORE // 2], mybir.dt.bfloat16)
            rhs_sb = wpool.tile([2 * KROWS, M], mybir.dt.bfloat16)
            # Chunked input DMAs: the first matmuls only depend on the first
            # chunks (Tile sub-tile deps), cutting ~12us of startup serial DMA.
            nc.sync.dma_start(out=lhsT_sb[:, 0:128], in_=lhsT_d[:, 0:128])
            nc.sync.dma_start(out=rhs_sb[:, 0:512], in_=rhs_d[:, 0:512])
            nc.sync.dma_start(out=rhs_sb[:, 512:1024], in_=rhs_d[:, 512:1024])
            for k in range(1, 8):
                nc.sync.dma_start(
                    out=rhs_sb[:, k * 1024:(k + 1) * 1024],
                    in_=rhs_d[:, k * 1024:(k + 1) * 1024],
                )
            nc.sync.dma_start(out=lhsT_sb[:, 128:2048], in_=lhsT_d[:, 128:2048])

            # racc[:, b] = per-source max of -D^2 over ALL targets for block b.
            racc = wpool.tile([128, NBLK], mybir.dt.float32)

            # Per-target accumulator strip over all 4 supertiles.
            A = wpool.tile([128, M], F16, name="A", tag="A")

            NGRP = NBLK // 2
            for g in range(NGRP):
                strips = [
                    epool.tile([128, M], F16, name=f"strip{i}", tag=f"strip{i}", bufs=4)
                    for i in range(2)
                ]
                for c in range(NSUP):
                    ps = [
                        pspool.tile([128, 2048], mybir.dt.float32, name=f"ps{i}", tag=f"ps{i}", bufs=1)
                        for i in range(2)
                    ]
                    for q in range(4):
                        j = c * 4 + q
                        for i in range(2):
                            nc.tensor.matmul(
                                ps[i][:, q * 512:(q + 1) * 512],
                                lhsT_sb[32 * i:32 * (i + 1), g * 128:(g + 1) * 128],
                                rhs_sb[32 * i:32 * (i + 1), j * 512:(j + 1) * 512],
                                start=True,
                                stop=True,
                            )
                    # Evacuate negated (-D^2) so both reductions are max ops.
                    for i in range(2):
                        nc.scalar.activation(
                            strips[i][:, c * 2048:(c + 1) * 2048],
                            ps[i],
                            mybir.ActivationFunctionType.Copy,
                            scale=-1.0,
                        )
                        if g == 0:
                            # Copy-init/fold per evac so DVE starts without
                            # waiting for the second strip (copy runs at 4x).
                            cs0 = slice(c * 2048, (c + 1) * 2048)
                            if i == 0:
                                nc.vector.tensor_copy(A[:, cs0], strips[0][:, cs0])
                            else:
                                nc.vector.tensor_tensor(A[:, cs0], A[:, cs0], strips[1][:, cs0], FMAX)

                    if 0 < g < 4 or g == NGRP - 1:
                        # Pipeline-fill (and pipeline-drain for the last
                        # group): fold per supertile so DVE starts right
                        # after each evac pair instead of per strip.
                        cs = slice(c * 2048, (c + 1) * 2048)
                        nc.vector.tensor_tensor(A[:, cs], A[:, cs], strips[0][:, cs], FMAX)
                        nc.vector.tensor_tensor(A[:, cs], A[:, cs], strips[1][:, cs], FMAX)

                # Fold into the per-target accumulator (fp16 TT at 2x, FD=8192).
                if 3 < g < NGRP - 1:
                    nc.vector.tensor_tensor(A, A, strips[0], FMAX)
                    nc.vector.tensor_tensor(A, A, strips[1], FMAX)

                # Forward: ONE fused custom-DVE op per strip -- in-place
                # max of the halves plus MAX-accum of the whole row into racc.
                for i in range(2):
                    b = 2 * g + i
                    s = strips[i]
                    nc.vector._custom_dve(
                        ttmax,
                        out=s[:, 0:4096],
                        in0=s[:, 0:4096],
                        in1=s[:, 4096:8192],
                        accum_out=racc[:, b:b + 1],
                    )

            nc.sync.dma_start(out=t2s_d[:, :], in_=A)
            nc.sync.dma_start(out=s2t_d[:, :], in_=racc)

    nc.compile()
    return nc


def _make_in_maps(source, target):
    """Build per-core input dicts (packed lhsT pairs + 2x-replicated rhs)."""
    planes = [_build_planes(source[b], target[b]) for b in range(B)]
    in_maps = []
    for i in range(NCORES):
        b, half = i // 2, i % 2
        L, R = planes[b]
        Lh = L[:, half * SRC_PER_CORE:(half + 1) * SRC_PER_CORE]
        L2 = np.zeros((2 * KROWS, SRC_PER_CORE // 2), BF16)
        for g in range(NBLK // 2):
            for j in range(2):
                L2[32 * j:32 * (j + 1), g * 128:(g + 1) * 128] = \
                    Lh[:, (2 * g + j) * 128:(2 * g + j + 1) * 128]
        R2 = np.concatenate([R, R], axis=0)
        in_maps.append({
            "lhsT": np.ascontiguousarray(L2),
            "rhs": np.ascontiguousarray(R2),
        })
    return in_maps


def _get_program():
    global _PROGRAM
    if _PROGRAM is None:
        _PROGRAM = _build_program()
    return _PROGRAM


def kernel(source, target, weights):
    from concourse.bass_utils import run_bass_kernel_spmd

    source = np.asarray(source)
    target = np.asarray(target)
    weights = np.asarray(weights)

    in_maps = _make_in_maps(source, target)

    nc = _get_program()
    res = None
    last_err = None
    for attempt in range(3):
        try:
            res = run_bass_kernel_spmd(nc, in_maps, list(range(NCORES))).results
            break
        except Exception as e:  # transient device wedge: retry
            last_err = e
            import time as _time

            _time.sleep(5.0 * (attempt + 1))
    if res is None:
        raise last_err

    s_minsq = np.empty((B, N), np.float64)
    t_minsq = np.empty((B, M), np.float64)
    for b in range(B):
        # s2t [128, 32] holds -min D^2: source n = blk*128 + p
        lo = -res[2 * b]["s2t"].astype(np.float64).T.reshape(-1)
        hi = -res[2 * b + 1]["s2t"].astype(np.float64).T.reshape(-1)
        s_minsq[b] = np.maximum(np.concatenate([lo, hi]), 0.0)
        fold = np.maximum(
            res[2 * b]["t2s"].astype(np.float64),
            res[2 * b + 1]["t2s"].astype(np.float64),
        )
        t_minsq[b] = np.maximum(-fold.max(0), 0.0)

    fwd = float((np.sqrt(s_minsq + EPS) * weights.astype(np.float64)).mean())
    bwd = float(np.sqrt(t_minsq + EPS).mean())
    return np.float32(fwd + bwd)


# revision 8
# speedup vs baseline: 1.1723x; 1.1723x over previous
"""Chamfer distance (symmetric, weighted forward) on 8 Trainium2 NeuronCores.

Strategy
--------
Brute-force all-pairs squared distances on the TensorEngine via the augmented
matmul  ||s||^2 + ||t||^2 - 2 s.t  with every fp32 operand split into 3 bf16
planes (products of bf16 planes are exact in fp32), so the PE computes
fp32-accurate squared distances at full bf16 streaming speed.

Sharding: 2 cores per batch element (B=4), each core takes 4096 of the 8192
source rows x all 8192 targets.  Within a core, source blocks are processed in
PAIRS whose K=32 weight sets sit at PE partition strips 0-31 / 32-63 (row-group
packed matmuls), so PE work never gates the pipeline.

Post-matmul pipeline:
  - ACT evacuates each PSUM tile [128, 2048] to SBUF fp16 with scale=-1
    (1x rate, ~2.36us measured) into a per-block [128, 8192] strip.  ACT and
    DVE are the only engines that can read PSUM (DMA has no PSUM route), so
    the evac rate bounds everything at ~300us/core of ACT time.
  - Backward (per-target): DVE folds each strip into the accumulator
    A [128, 8192] with one FD-8192 fp16 tensor_tensor max (2x_1P, ~5.35us);
    31 folds total -- the minimum.
  - Forward (per-source): ONE custom DVE op per strip (TT_MAX_REDUCE_CHAMFER,
    registered at import via the documented dve_ops.OPS extension point):
    body = max(Src0, Src1) over the strip halves (in-place), with
    accum=MAX writing the full row-max straight to racc[:, b].  This replaces
    the old 6-instruction TT pyramid + batched tensor_reduce (~6.9us/strip)
    with a single ~4.6us 1x instruction, taking DVE off the critical path.
  - Input DMAs are chunked so the first matmuls start ~11us in instead of 16.
Dead ends (hardware-verified): tensor_tensor_reduce and non-contiguous 3D-AP
TTs fault the DVE via this runtime; pool/tensor_reduce/max8 have only 1x uops;
GpSimd rejects TENSOR_TENSOR at codegen and its tensor_reduce is C-axis only.

The host builds the bf16 split planes, runs the SPMD kernel, folds the
[128 x 32-block] partition structure, takes sqrt and means.  fp16 rounding of
squared distances costs ~2.4e-4 relative on the result -- far inside the
2e-2 gate (measured end-to-end rel err 6e-7).
"""

import os
import sys

import numpy as np

for _p in ("/root/.axon_site", "/root/.axon_site/_ro/trn_rl_repo", "/root/.axon_site/_ro/pypackages"):
    if os.path.isdir(_p) and _p not in sys.path:
        sys.path.append(_p)

import ml_dtypes

BF16 = ml_dtypes.bfloat16

# Problem constants (hardcoded per spec)
B = 4
N = 8192  # sources per batch
M = 8192  # targets per batch
NCORES = 8
SRC_PER_CORE = N // 2        # 4096
NBLK = SRC_PER_CORE // 128   # 32 source blocks per core
NSUP = M // 2048             # 4 target supertiles per batch
KROWS = 32
EPS = 1e-8

_PROGRAM = None  # cached (nc, ...) build
_TTMAX = None    # cached custom DVE op


def _get_ttmax_op():
    """Register (once) the fused max+row-max-reduce custom DVE op.

    body: out[k] = max(in0[k], in1[k]); accum_out = max_k out[k]
    (accum seeded with the MAX identity -FLT_MAX).
    Registered via the documented extension point (dve_ops.OPS append); the
    per-NEFF uop table is generated from the module-level OPS list at
    compile time, so this must run before nc.compile().
    """
    global _TTMAX
    if _TTMAX is not None:
        return _TTMAX
    from concourse import dve_ops as DO

    name = "TT_MAX_REDUCE_CHAMFER"
    for op in DO.OPS:
        if op.name == name:
            _TTMAX = op
            return op

    from concourse.dve_spec import Spec, Src0, Src1, lower, maxx
    from concourse.dve_uop import DveOpSpec

    def _ref(in0, in1, s0, s1, imm2):
        b = np.maximum(in0.astype(np.float32), in1.astype(np.float32))
        return b, b.reshape(b.shape[0], -1).max(-1, keepdims=True)

    spec = Spec(body=maxx(Src0, Src1), accum=maxx, reference=_ref)
    # Pin the sha from this very build (no external golden needed).
    shas = {
        ver: DveOpSpec(name=name, opcode=0x1F, uops=lower(spec, ver=ver), rd1_en=True).sha(ver)
        for ver in ("v3", "v4")
    }
    op = DO.DveOp(name, spec, subdim=False, uops_sha=shas)
    row = DO._CUSTOM_DVE_ROW_BASE + len(DO.OPS)
    assert row < 0x20, "custom-DVE opcode row overflow"
    DO.OPS.append(op)
    DO.CUSTOM_DVE_SPECS[name] = spec
    DO._SUB_OPCODE_FOR_NAME[name] = row
    _TTMAX = op
    return op


def _splitn(x, n):
    """Split fp64 array into n bf16 planes summing (to ~8n bits) to x."""
    x = x.astype(np.float64)
    out = []
    for _ in range(n):
        a = x.astype(BF16)
        out.append(a)
        x = x - a.astype(np.float64)
    return out


def _build_planes(src_b, tgt_b):
    """Augmented K=32 bf16 planes for one batch.

    Returns L [32, N] (source side / lhsT) and R [32, M] (target side / rhs)
    such that sum_k L[k, n] * R[k, m] == ||s_n - t_m||^2 up to fp32 rounding.
    """
    sa, sb, sc = _splitn(-2.0 * src_b.astype(np.float64), 3)  # (N, 3) each
    ta, tb, tc = _splitn(tgt_b.astype(np.float64), 3)
    ns = (src_b.astype(np.float64) ** 2).sum(1)
    nt = (tgt_b.astype(np.float64) ** 2).sum(1)
    nss = _splitn(ns, 4)
    nts = _splitn(nt, 4)
    one_s = np.ones(ns.shape, BF16)
    one_t = np.ones(nt.shape, BF16)
    Ls, Rs = [], []
    for k in range(3):
        # products: ad ae af bd be bf cd ce (only c*f dropped, ~2^-32 rel)
        for (u, v) in [(sa, ta), (sa, tb), (sa, tc), (sb, ta), (sb, tb), (sb, tc), (sc, ta), (sc, tb)]:
            Ls.append(u[:, k])
            Rs.append(v[:, k])
    for u in nss:
        Ls.append(u)
        Rs.append(one_t)
    for v in nts:
        Ls.append(one_s)
        Rs.append(v)
    L = np.ascontiguousarray(np.stack(Ls, 0).astype(BF16))
    R = np.ascontiguousarray(np.stack(Rs, 0).astype(BF16))
    return L, R


def _build_program():
    """Build the SPMD Tile program once. Returns the finalized Bass object."""
    import concourse.bacc as bacc
    import concourse.tile as tile
    from concourse import mybir

    ttmax = _get_ttmax_op()

    nc = bacc.Bacc("TRN2", target_bir_lowering=False, debug=False, num_devices=NCORES)

    # lhsT packs PAIRS of 128-source blocks at partition strips 0-31 / 32-63
    # so the two K=32 matmuls run concurrently on distinct PE row-groups.
    lhsT_d = nc.dram_tensor("lhsT", [2 * KROWS, SRC_PER_CORE // 2], mybir.dt.bfloat16, kind="ExternalInput")
    rhs_d = nc.dram_tensor("rhs", [2 * KROWS, M], mybir.dt.bfloat16, kind="ExternalInput")
    s2t_d = nc.dram_tensor("s2t", [128, NBLK], mybir.dt.float32, kind="ExternalOutput")
    t2s_d = nc.dram_tensor("t2s", [128, M], mybir.dt.float16, kind="ExternalOutput")

    FMAX = mybir.AluOpType.max
    F16 = mybir.dt.float16

    with tile.TileContext(nc) as tc:
        with (
            tc.tile_pool(name="weights", bufs=1) as wpool,
            tc.tile_pool(name="psum", bufs=2, space="PSUM") as pspool,
            tc.tile_pool(name="evac", bufs=12) as epool,
        ):
            lhsT_sb = wpool.tile([2 * KROWS, SRC_PER_CORE // 2], mybir.dt.bfloat16)
            rhs_sb = wpool.tile([2 * KROWS, M], mybir.dt.bfloat16)
            # Chunked input DMAs: the first matmuls only depend on the first
            # chunks (Tile sub-tile deps), cutting ~12us of startup serial DMA.
            nc.sync.dma_start(out=lhsT_sb[:, 0:128], in_=lhsT_d[:, 0:128])
            nc.sync.dma_start(out=rhs_sb[:, 0:256], in_=rhs_d[:, 0:256])
            nc.sync.dma_start(out=rhs_sb[:, 256:1024], in_=rhs_d[:, 256:1024])
            for k in range(1, 8):
                nc.sync.dma_start(
                    out=rhs_sb[:, k * 1024:(k + 1) * 1024],
                    in_=rhs_d[:, k * 1024:(k + 1) * 1024],
                )
            nc.sync.dma_start(out=lhsT_sb[:, 128:2048], in_=lhsT_d[:, 128:2048])

            # racc[:, b] = per-source max of -D^2 over ALL targets for block b.
            racc = wpool.tile([128, NBLK], mybir.dt.float32)

            # Per-target accumulator strip over all 4 supertiles.
            A = wpool.tile([128, M], F16, name="A", tag="A")

            NGRP = NBLK // 2
            for g in range(NGRP):
                strips = [
                    epool.tile([128, M], F16, name=f"strip{i}", tag=f"strip{i}", bufs=3)
                    for i in range(2)
                ]
                for c in range(NSUP):
                    ps = [
                        pspool.tile([128, 2048], mybir.dt.float32, name=f"ps{i}", tag=f"ps{i}", bufs=1)
                        for i in range(2)
                    ]
                    for q in range(4):
                        j = c * 4 + q
                        for i in range(2):
                            nc.tensor.matmul(
                                ps[i][:, q * 512:(q + 1) * 512],
                                lhsT_sb[32 * i:32 * (i + 1), g * 128:(g + 1) * 128],
                                rhs_sb[32 * i:32 * (i + 1), j * 512:(j + 1) * 512],
                                start=True,
                                stop=True,
                            )
                    # Evacuate negated (-D^2) so both reductions are max ops.
                    for i in range(2):
                        nc.scalar.activation(
                            strips[i][:, c * 2048:(c + 1) * 2048],
                            ps[i],
                            mybir.ActivationFunctionType.Copy,
                            scale=-1.0,
                        )
                        if g == 0:
                            # Copy-init/fold per evac so DVE starts without
                            # waiting for the second strip (copy runs at 4x).
                            cs0 = slice(c * 2048, (c + 1) * 2048)
                            if i == 0:
                                nc.vector.tensor_copy(A[:, cs0], strips[0][:, cs0])
                            else:
                                nc.vector.tensor_tensor(A[:, cs0], A[:, cs0], strips[1][:, cs0], FMAX)

                    if g == NGRP - 1:
                        # Pipeline-drain for the last group: fold per
                        # supertile so the tail after the final evac is
                        # only the custom ops.
                        cs = slice(c * 2048, (c + 1) * 2048)
                        nc.vector.tensor_tensor(A[:, cs], A[:, cs], strips[0][:, cs], FMAX)
                        nc.vector.tensor_tensor(A[:, cs], A[:, cs], strips[1][:, cs], FMAX)

                # Fold into the per-target accumulator (fp16 TT at 2x, FD=8192).
                if 0 < g < NGRP - 1:
                    nc.vector.tensor_tensor(A, A, strips[0], FMAX)
                    nc.vector.tensor_tensor(A, A, strips[1], FMAX)

                # Forward: ONE fused custom-DVE op per strip -- in-place
                # max of the halves plus MAX-accum of the whole row into racc.
                for i in range(2):
                    b = 2 * g + i
                    s = strips[i]
                    nc.vector._custom_dve(
                        ttmax,
                        out=s[:, 0:4096],
                        in0=s[:, 0:4096],
                        in1=s[:, 4096:8192],
                        accum_out=racc[:, b:b + 1],
                    )

            nc.sync.dma_start(out=t2s_d[:, :], in_=A)
            nc.sync.dma_start(out=s2t_d[:, :], in_=racc)

    nc.compile()
    return nc


def _make_in_maps(source, target):
    """Build per-core input dicts (packed lhsT pairs + 2x-replicated rhs)."""
    planes = [_build_planes(source[b], target[b]) for b in range(B)]
    in_maps = []
    for i in range(NCORES):
        b, half = i // 2, i % 2
        L, R = planes[b]
        Lh = L[:, half * SRC_PER_CORE:(half + 1) * SRC_PER_CORE]
        L2 = np.zeros((2 * KROWS, SRC_PER_CORE // 2), BF16)
        for g in range(NBLK // 2):
            for j in range(2):
                L2[32 * j:32 * (j + 1), g * 128:(g + 1) * 128] = \
                    Lh[:, (2 * g + j) * 128:(2 * g + j + 1) * 128]
        R2 = np.concatenate([R, R], axis=0)
        in_maps.append({
            "lhsT": np.ascontiguousarray(L2),
            "rhs": np.ascontiguousarray(R2),
        })
    return in_maps


def _get_program():
    global _PROGRAM
    if _PROGRAM is None:
        _PROGRAM = _build_program()
    return _PROGRAM


def kernel(source, target, weights):
    from concourse.bass_utils import run_bass_kernel_spmd

    source = np.asarray(source)
    target = np.asarray(target)
    weights = np.asarray(weights)

    in_maps = _make_in_maps(source, target)

    nc = _get_program()
    res = None
    last_err = None
    for attempt in range(3):
        try:
            res = run_bass_kernel_spmd(nc, in_maps, list(range(NCORES))).results
            break
        except Exception as e:  # transient device wedge: retry
            last_err = e
            import time as _time

            _time.sleep(5.0 * (attempt + 1))
    if res is None:
        raise last_err

    s_minsq = np.empty((B, N), np.float64)
    t_minsq = np.empty((B, M), np.float64)
    for b in range(B):
        # s2t [128, 32] holds -min D^2: source n = blk*128 + p
        lo = -res[2 * b]["s2t"].astype(np.float64).T.reshape(-1)
        hi = -res[2 * b + 1]["s2t"].astype(np.float64).T.reshape(-1)
        s_minsq[b] = np.maximum(np.concatenate([lo, hi]), 0.0)
        fold = np.maximum(
            res[2 * b]["t2s"].astype(np.float64),
            res[2 * b + 1]["t2s"].astype(np.float64),
        )
        t_minsq[b] = np.maximum(-fold.max(0), 0.0)

    fwd = float((np.sqrt(s_minsq + EPS) * weights.astype(np.float64)).mean())
    bwd = float(np.sqrt(t_minsq + EPS).mean())
    return np.float32(fwd + bwd)


# revision 10
# speedup vs baseline: 1.1954x; 1.0197x over previous
"""Chamfer distance (symmetric, weighted forward) on 8 Trainium2 NeuronCores.

Strategy
--------
Brute-force all-pairs squared distances on the TensorEngine via the augmented
matmul  ||s||^2 + ||t||^2 - 2 s.t  with every fp32 operand split into 3 bf16
planes (products of bf16 planes are exact in fp32), so the PE computes
fp32-accurate squared distances at full bf16 streaming speed.

Sharding: 2 cores per batch element (B=4), each core takes 4096 of the 8192
source rows x all 8192 targets.  Within a core, source blocks are processed in
PAIRS whose K=32 weight sets sit at PE partition strips 0-31 / 32-63 (row-group
packed matmuls), so PE work never gates the pipeline.

Post-matmul pipeline:
  - ACT evacuates each PSUM tile [128, 2048] to SBUF fp16 with scale=-1
    (1x rate, ~2.36us measured) into a per-block [128, 8192] strip.  ACT and
    DVE are the only engines that can read PSUM (DMA has no PSUM route), so
    the evac rate bounds everything at ~300us/core of ACT time.
  - Backward (per-target): DVE folds each strip into the accumulator
    A [128, 8192] with one FD-8192 fp16 tensor_tensor max (2x_1P, ~5.35us);
    31 folds total -- the minimum.
  - Forward (per-source): ONE custom DVE op per strip (TT_MAX_REDUCE_CHAMFER,
    registered at import via the documented dve_ops.OPS extension point):
    body = max(Src0, Src1) over the strip halves (in-place), with
    accum=MAX writing the full row-max straight to racc[:, b].  This replaces
    the old 6-instruction TT pyramid + batched tensor_reduce (~6.9us/strip)
    with a single ~4.6us 1x instruction, taking DVE off the critical path.
  - Input DMAs are chunked so the first matmuls start ~11us in instead of 16.
Dead ends (hardware-verified): tensor_tensor_reduce and non-contiguous 3D-AP
TTs fault the DVE via this runtime; pool/tensor_reduce/max8 have only 1x uops;
GpSimd rejects TENSOR_TENSOR at codegen and its tensor_reduce is C-axis only.

The host builds the bf16 split planes, runs the SPMD kernel, folds the
[128 x 32-block] partition structure, takes sqrt and means.  fp16 rounding of
squared distances costs ~2.4e-4 relative on the result -- far inside the
2e-2 gate (measured end-to-end rel err 6e-7).
"""

import os
import sys

import numpy as np

for _p in ("/root/.axon_site", "/root/.axon_site/_ro/trn_rl_repo", "/root/.axon_site/_ro/pypackages"):
    if os.path.isdir(_p) and _p not in sys.path:
        sys.path.append(_p)

import ml_dtypes

BF16 = ml_dtypes.bfloat16

# Problem constants (hardcoded per spec)
B = 4
N = 8192  # sources per batch
M = 8192  # targets per batch
NCORES = 8
SRC_PER_CORE = N // 2        # 4096
NBLK = SRC_PER_CORE // 128   # 32 source blocks per core
NSUP = M // 2048             # 4 target supertiles per batch
KROWS = 32
EPS = 1e-8

_PROGRAM = None  # cached (nc, ...) build
_TTMAX = None    # cached custom DVE op


def _get_ttmax_op():
    """Register (once) the fused max+row-max-reduce custom DVE op.

    body: out[k] = max(in0[k], in1[k]); accum_out = max_k out[k]
    (accum seeded with the MAX identity -FLT_MAX).
    Registered via the documented extension point (dve_ops.OPS append); the
    per-NEFF uop table is generated from the module-level OPS list at
    compile time, so this must run before nc.compile().
    """
    global _TTMAX
    if _TTMAX is not None:
        return _TTMAX
    from concourse import dve_ops as DO

    name = "TT_MAX_REDUCE_CHAMFER"
    for op in DO.OPS:
        if op.name == name:
            _TTMAX = op
            return op

    from concourse.dve_spec import Spec, Src0, Src1, lower, maxx
    from concourse.dve_uop import DveOpSpec

    def _ref(in0, in1, s0, s1, imm2):
        b = np.maximum(in0.astype(np.float32), in1.astype(np.float32))
        return b, b.reshape(b.shape[0], -1).max(-1, keepdims=True)

    spec = Spec(body=maxx(Src0, Src1), accum=maxx, reference=_ref)
    # Pin the sha from this very build (no external golden needed).
    shas = {
        ver: DveOpSpec(name=name, opcode=0x1F, uops=lower(spec, ver=ver), rd1_en=True).sha(ver)
        for ver in ("v3", "v4")
    }
    op = DO.DveOp(name, spec, subdim=False, uops_sha=shas)
    row = DO._CUSTOM_DVE_ROW_BASE + len(DO.OPS)
    assert row < 0x20, "custom-DVE opcode row overflow"
    DO.OPS.append(op)
    DO.CUSTOM_DVE_SPECS[name] = spec
    DO._SUB_OPCODE_FOR_NAME[name] = row
    _TTMAX = op
    return op


def _splitn(x, n):
    """Split fp64 array into n bf16 planes summing (to ~8n bits) to x."""
    x = x.astype(np.float64)
    out = []
    for _ in range(n):
        a = x.astype(BF16)
        out.append(a)
        x = x - a.astype(np.float64)
    return out


def _build_planes(src_b, tgt_b):
    """Augmented K=32 bf16 planes for one batch.

    Returns L [32, N] (source side / lhsT) and R [32, M] (target side / rhs)
    such that sum_k L[k, n] * R[k, m] == ||s_n - t_m||^2 up to fp32 rounding.
    """
    sa, sb, sc = _splitn(-2.0 * src_b.astype(np.float64), 3)  # (N, 3) each
    ta, tb, tc = _splitn(tgt_b.astype(np.float64), 3)
    ns = (src_b.astype(np.float64) ** 2).sum(1)
    nt = (tgt_b.astype(np.float64) ** 2).sum(1)
    nss = _splitn(ns, 4)
    nts = _splitn(nt, 4)
    one_s = np.ones(ns.shape, BF16)
    one_t = np.ones(nt.shape, BF16)
    Ls, Rs = [], []
    for k in range(3):
        # products: ad ae af bd be bf cd ce (only c*f dropped, ~2^-32 rel)
        for (u, v) in [(sa, ta), (sa, tb), (sa, tc), (sb, ta), (sb, tb), (sb, tc), (sc, ta), (sc, tb)]:
            Ls.append(u[:, k])
            Rs.append(v[:, k])
    for u in nss:
        Ls.append(u)
        Rs.append(one_t)
    for v in nts:
        Ls.append(one_s)
        Rs.append(v)
    L = np.ascontiguousarray(np.stack(Ls, 0).astype(BF16))
    R = np.ascontiguousarray(np.stack(Rs, 0).astype(BF16))
    return L, R


def _build_program():
    """Build the SPMD Tile program once. Returns the finalized Bass object."""
    import concourse.bacc as bacc
    import concourse.tile as tile
    from concourse import mybir

    ttmax = _get_ttmax_op()

    nc = bacc.Bacc("TRN2", target_bir_lowering=False, debug=False, num_devices=NCORES)

    # lhsT packs PAIRS of 128-source blocks at partition strips 0-31 / 32-63
    # so the two K=32 matmuls run concurrently on distinct PE row-groups.
    lhsT_d = nc.dram_tensor("lhsT", [2 * KROWS, SRC_PER_CORE // 2], mybir.dt.bfloat16, kind="ExternalInput")
    rhs_d = nc.dram_tensor("rhs", [2 * KROWS, M], mybir.dt.bfloat16, kind="ExternalInput")
    s2t_d = nc.dram_tensor("s2t", [128, NBLK], mybir.dt.float32, kind="ExternalOutput")
    t2s_d = nc.dram_tensor("t2s", [128, M], mybir.dt.float16, kind="ExternalOutput")

    FMAX = mybir.AluOpType.max
    F16 = mybir.dt.float16

    with tile.TileContext(nc) as tc:
        with (
            tc.tile_pool(name="weights", bufs=1) as wpool,
            tc.tile_pool(name="psum", bufs=2, space="PSUM") as pspool,
            tc.tile_pool(name="evac", bufs=12) as epool,
        ):
            lhsT_sb = wpool.tile([2 * KROWS, SRC_PER_CORE // 2], mybir.dt.bfloat16)
            rhs_sb = wpool.tile([2 * KROWS, M], mybir.dt.bfloat16)
            # Chunked input DMAs: the first matmuls only depend on the first
            # chunks (Tile sub-tile deps), cutting ~12us of startup serial DMA.
            nc.sync.dma_start(out=lhsT_sb[:, 0:256], in_=lhsT_d[:, 0:256])
            for k in range(8):
                nc.sync.dma_start(
                    out=rhs_sb[:, k * 1024:(k + 1) * 1024],
                    in_=rhs_d[:, k * 1024:(k + 1) * 1024],
                )
            nc.sync.dma_start(out=lhsT_sb[:, 256:2048], in_=lhsT_d[:, 256:2048])

            # racc[:, b] = per-source max of -D^2 over ALL targets for block b.
            racc = wpool.tile([128, NBLK], mybir.dt.float32)

            # Per-target accumulator strip over all 4 supertiles.
            A = wpool.tile([128, M], F16, name="A", tag="A")

            NGRP = NBLK // 2
            for g in range(NGRP):
                strips = [
                    epool.tile([128, M], F16, name=f"strip{i}", tag=f"strip{i}", bufs=3)
                    for i in range(2)
                ]
                for c in range(NSUP):
                    ps = [
                        pspool.tile([128, 2048], mybir.dt.float32, name=f"ps{i}", tag=f"ps{i}", bufs=1)
                        for i in range(2)
                    ]
                    for q in range(4):
                        j = c * 4 + q
                        for i in range(2):
                            nc.tensor.matmul(
                                ps[i][:, q * 512:(q + 1) * 512],
                                lhsT_sb[32 * i:32 * (i + 1), g * 128:(g + 1) * 128],
                                rhs_sb[32 * i:32 * (i + 1), j * 512:(j + 1) * 512],
                                start=True,
                                stop=True,
                            )
                    # Evacuate negated (-D^2) so both reductions are max ops.
                    for i in range(2):
                        nc.scalar.activation(
                            strips[i][:, c * 2048:(c + 1) * 2048],
                            ps[i],
                            mybir.ActivationFunctionType.Copy,
                            scale=-1.0,
                        )
                        if g == 0:
                            # Copy-init/fold per evac so DVE starts without
                            # waiting for the second strip (copy runs at 4x).
                            cs0 = slice(c * 2048, (c + 1) * 2048)
                            if i == 0:
                                nc.vector.tensor_copy(A[:, cs0], strips[0][:, cs0])
                            else:
                                nc.vector.tensor_tensor(A[:, cs0], A[:, cs0], strips[1][:, cs0], FMAX)

                    if 0 < g < 4:
                        # Pipeline-fill: fold per supertile so DVE starts
                        # right after each evac pair instead of per strip.
                        cs = slice(c * 2048, (c + 1) * 2048)
                        nc.vector.tensor_tensor(A[:, cs], A[:, cs], strips[0][:, cs], FMAX)
                        nc.vector.tensor_tensor(A[:, cs], A[:, cs], strips[1][:, cs], FMAX)

                # Fold into the per-target accumulator (fp16 TT at 2x, FD=8192).
                if g > 3:
                    nc.vector.tensor_tensor(A, A, strips[0], FMAX)
                    nc.vector.tensor_tensor(A, A, strips[1], FMAX)

                # Forward: ONE fused custom-DVE op per strip -- in-place
                # max of the halves plus MAX-accum of the whole row into racc.
                for i in range(2):
                    b = 2 * g + i
                    s = strips[i]
                    nc.vector._custom_dve(
                        ttmax,
                        out=s[:, 0:4096],
                        in0=s[:, 0:4096],
                        in1=s[:, 4096:8192],
                        accum_out=racc[:, b:b + 1],
                    )

            nc.sync.dma_start(out=t2s_d[:, :], in_=A)
            nc.sync.dma_start(out=s2t_d[:, :], in_=racc)

    nc.compile()
    return nc


def _make_in_maps(source, target):
    """Build per-core input dicts (packed lhsT pairs + 2x-replicated rhs)."""
    planes = [_build_planes(source[b], target[b]) for b in range(B)]
    in_maps = []
    for i in range(NCORES):
        b, half = i // 2, i % 2
        L, R = planes[b]
        Lh = L[:, half * SRC_PER_CORE:(half + 1) * SRC_PER_CORE]
        L2 = np.zeros((2 * KROWS, SRC_PER_CORE // 2), BF16)
        for g in range(NBLK // 2):
            for j in range(2):
                L2[32 * j:32 * (j + 1), g * 128:(g + 1) * 128] = \
                    Lh[:, (2 * g + j) * 128:(2 * g + j + 1) * 128]
        R2 = np.concatenate([R, R], axis=0)
        in_maps.append({
            "lhsT": np.ascontiguousarray(L2),
            "rhs": np.ascontiguousarray(R2),
        })
    return in_maps


def _get_program():
    global _PROGRAM
    if _PROGRAM is None:
        _PROGRAM = _build_program()
    return _PROGRAM


def kernel(source, target, weights):
    from concourse.bass_utils import run_bass_kernel_spmd

    source = np.asarray(source)
    target = np.asarray(target)
    weights = np.asarray(weights)

    in_maps = _make_in_maps(source, target)

    nc = _get_program()
    res = None
    last_err = None
    for attempt in range(3):
        try:
            res = run_bass_kernel_spmd(nc, in_maps, list(range(NCORES))).results
            break
        except Exception as e:  # transient device wedge: retry
            last_err = e
            import time as _time

            _time.sleep(5.0 * (attempt + 1))
    if res is None:
        raise last_err

    s_minsq = np.empty((B, N), np.float64)
    t_minsq = np.empty((B, M), np.float64)
    for b in range(B):
        # s2t [128, 32] holds -min D^2: source n = blk*128 + p
        lo = -res[2 * b]["s2t"].astype(np.float64).T.reshape(-1)
        hi = -res[2 * b + 1]["s2t"].astype(np.float64).T.reshape(-1)
        s_minsq[b] = np.maximum(np.concatenate([lo, hi]), 0.0)
        fold = np.maximum(
            res[2 * b]["t2s"].astype(np.float64),
            res[2 * b + 1]["t2s"].astype(np.float64),
        )
        t_minsq[b] = np.maximum(-fold.max(0), 0.0)

    fwd = float((np.sqrt(s_minsq + EPS) * weights.astype(np.float64)).mean())
    bwd = float(np.sqrt(t_minsq + EPS).mean())
    return np.float32(fwd + bwd)


# revision 12
# speedup vs baseline: 1.3005x; 1.0879x over previous
"""Chamfer distance (symmetric, weighted forward) on 8 Trainium2 NeuronCores.

Strategy
--------
Brute-force all-pairs squared distances on the TensorEngine via the augmented
matmul  ||s||^2 + ||t||^2 - 2 s.t  with every fp32 operand split into 3 bf16
planes (products of bf16 planes are exact in fp32), so the PE computes
fp32-accurate squared distances at full bf16 streaming speed.

Sharding: 2 cores per batch element (B=4), each core takes 4096 of the 8192
source rows x all 8192 targets.  Within a core, source blocks are processed in
PAIRS whose K=32 weight sets sit at PE partition strips 0-31 / 32-63 (row-group
packed matmuls), so PE work never gates the pipeline.

Post-matmul pipeline:
  - ACT evacuates each PSUM tile [128, 2048] to SBUF fp16 with scale=-1
    (1x rate, ~2.36us measured) into a per-block [128, 8192] strip.  ACT and
    DVE are the only engines that can read PSUM (DMA has no PSUM route), so
    the evac rate bounds everything at ~300us/core of ACT time.
  - Backward (per-target): DVE folds each strip into the accumulator
    A [128, 8192] with one FD-8192 fp16 tensor_tensor max (2x_1P, ~5.35us);
    31 folds total -- the minimum.
  - Forward (per-source): ONE custom DVE op per strip (TT_MAX_REDUCE_CHAMFER,
    registered at import via the documented dve_ops.OPS extension point):
    body = max(Src0, Src1) over the strip halves (in-place), with
    accum=MAX writing the full row-max straight to racc[:, b].  This replaces
    the old 6-instruction TT pyramid + batched tensor_reduce (~6.9us/strip)
    with a single ~4.6us 1x instruction, taking DVE off the critical path.
  - Input DMAs are chunked so the first matmuls start ~11us in instead of 16.
Dead ends (hardware-verified): tensor_tensor_reduce and non-contiguous 3D-AP
TTs fault the DVE via this runtime; pool/tensor_reduce/max8 have only 1x uops;
GpSimd rejects TENSOR_TENSOR at codegen and its tensor_reduce is C-axis only.

The host builds the bf16 split planes, runs the SPMD kernel, folds the
[128 x 32-block] partition structure, takes sqrt and means.  fp16 rounding of
squared distances costs ~2.4e-4 relative on the result -- far inside the
2e-2 gate (measured end-to-end rel err 6e-7).
"""

import os
import sys

import numpy as np

for _p in ("/root/.axon_site", "/root/.axon_site/_ro/trn_rl_repo", "/root/.axon_site/_ro/pypackages"):
    if os.path.isdir(_p) and _p not in sys.path:
        sys.path.append(_p)

import ml_dtypes

BF16 = ml_dtypes.bfloat16

# Problem constants (hardcoded per spec)
B = 4
N = 8192  # sources per batch
M = 8192  # targets per batch
NCORES = 8
SRC_PER_CORE = N // 2        # 4096
NBLK = SRC_PER_CORE // 128   # 32 source blocks per core
NSUP = M // 2048             # 4 target supertiles per batch
KROWS = 32
EPS = 1e-8

_PROGRAM = None  # cached (nc, ...) build
_TTMAX = None    # cached custom DVE op


def _get_ttmax_op():
    """Register (once) the fused pairwise-max + running-row-max custom DVE op,
    with a hand-authored 2x_1P perf-mode uop program.

    Semantics (both the auto-lowered 1x REGULAR program and the 2x variant):
        out[k] = running max over elements 0..k of max(in0[.], in1[.])
    so out[:, -1] is the full row max of both inputs.  Expressing the
    reduction as a scan keeps the result in the normal output stream --
    no accum_out / accumulator-readout involved, which is what makes a
    perf-mode variant safe to author (the accumulator drain path is
    firmware-internal and undocumented).

    The 2x_1P program packs two fp16 elements per 32-bit port read
    (SRC_0/SRC_0_HI, SRC_1/SRC_1_HI), computes lo-max, hi-max, pair-max,
    then the scan state at stage 3, writing the state to both WR0_LO/HI.
    If the RTL declines 2x (any trigger condition unmet) it falls back to
    the REGULAR scan program -- same semantics at 1x, so this is safe.

    Registered via the documented extension point (dve_ops.OPS append); the
    per-NEFF uop table is generated from the module-level OPS list at
    compile time, so this must run before nc.compile().  The compile cache
    is pre-seeded with our DveOpSpec so the uops_2x variant survives
    DveOp.compile().
    """
    global _TTMAX
    if _TTMAX is not None:
        return _TTMAX
    import copy as _copy

    from concourse import dve_ops as DO

    name = "TT_SCANMAX2X_CHAMFER"
    for op in DO.OPS:
        if op.name == name:
            _TTMAX = op
            return op

    from concourse.dve_spec import Spec, Src0, Src1, lower, maxx, scan
    from concourse.dve_uop import (
        AluInp,
        AluOp,
        DelayInp,
        DveOpSpec,
        InpSel,
        OutPath,
        OutSel,
        UopDpConfig,
    )

    def _ref(in0, in1, s0, s1, imm2):
        b = np.maximum(in0.astype(np.float32), in1.astype(np.float32))
        return np.maximum.accumulate(b.reshape(b.shape[0], -1), axis=-1)

    spec = Spec(body=scan(AluOp.MAX, maxx(Src0, Src1)), reference=_ref)
    row = DO._CUSTOM_DVE_ROW_BASE + len(DO.OPS)
    assert row < 0x20, "custom-DVE opcode row overflow"

    ver = "v3"  # TRN2
    uops1x = lower(spec, ver=ver)
    seed2x = _copy.deepcopy(uops1x[0])
    steady2x = _copy.deepcopy(uops1x[1])
    for u in (seed2x, steady2x):
        # Input lanes: stage-0 PREV_DELAY_k = inp[k+1].
        u.inp = [InpSel.ZERO, InpSel.SRC_0, InpSel.SRC_1, InpSel.MAX_NEG,
                 InpSel.SRC_0_HI, InpSel.SRC_1_HI, InpSel.ZERO, InpSel.ZERO]
        u.inp_enable = [0, 1, 1, 1, 1, 1, 0, 0]

    P, D = DelayInp.PREV_ALU_OUT, DelayInp.PREV_DELAY

    def _dp(op=AluOp.BYPASS, s0=AluInp.PREV_ALU_OUT, s1=AluInp.PREV_ALU_OUT,
            delay=None, den=None):
        return UopDpConfig(
            op=op, alu_src0=s0, alu_src1=s1,
            delay=list(delay or [D, D, D, P, P, P, P]),
            alu_out_enable=1,
            delay_enable=list(den or [1, 1, 1, 0, 0, 0, 0]),
        )

    # Seed (1 cycle): route MAX_NEG (inp3 -> PREV_DELAY_2) to stage 3's out
    # flop, which the steady state reads back as CURR_ALU_OUT.
    seed2x.datapath_config = [
        _dp(), _dp(), _dp(),
        _dp(op=AluOp.BYPASS, s0=AluInp.PREV_DELAY_2, s1=AluInp.PREV_DELAY_2),
        _dp(), _dp(), _dp(), _dp(),
    ]
    # Steady: lo-max, hi-max, pair-max, scan state; state rides to the output.
    steady2x.datapath_config = [
        _dp(op=AluOp.MAX, s0=AluInp.PREV_DELAY_0, s1=AluInp.PREV_DELAY_1,
            delay=[D, D, D, D, D, P, P], den=[1, 1, 1, 1, 1, 0, 0]),
        _dp(op=AluOp.MAX, s0=AluInp.PREV_DELAY_3, s1=AluInp.PREV_DELAY_4,
            delay=[P, D, D, D, D, P, P], den=[1, 1, 1, 1, 1, 0, 0]),
        _dp(op=AluOp.MAX, s0=AluInp.PREV_ALU_OUT, s1=AluInp.PREV_DELAY_0),
        _dp(op=AluOp.MAX, s0=AluInp.CURR_ALU_OUT, s1=AluInp.PREV_ALU_OUT),
        _dp(), _dp(), _dp(), _dp(),
    ]
    steady2x.out = {OutPath.WR0_LO: OutSel.ALU_OUT, OutPath.WR0_HI: OutSel.ALU_OUT,
                    OutPath.WR1_LO: OutSel.ALU_OUT, OutPath.WR1_HI: OutSel.ALU_OUT}
    steady2x.out_enable = {OutPath.WR0_LO: 1, OutPath.WR0_HI: 1,
                           OutPath.WR1_LO: 0, OutPath.WR1_HI: 0}

    ospec = DveOpSpec(name=name, opcode=row, uops=uops1x,
                      uops_2x=[seed2x, steady2x], perf_max=1, rd1_en=True)
    ospec.validate(ver)
    op = DO.DveOp(name, spec, subdim=False, uops_sha={ver: ospec.sha(ver)})
    # Pre-seed the compile cache: DveOp.compile() would otherwise rebuild the
    # spec without the uops_2x variant.
    DO._COMPILE_CACHE[(name, ver)] = ospec
    DO.OPS.append(op)
    DO.CUSTOM_DVE_SPECS[name] = spec
    DO._SUB_OPCODE_FOR_NAME[name] = row
    _TTMAX = op
    return op


def _splitn(x, n):
    """Split fp64 array into n bf16 planes summing (to ~8n bits) to x."""
    x = x.astype(np.float64)
    out = []
    for _ in range(n):
        a = x.astype(BF16)
        out.append(a)
        x = x - a.astype(np.float64)
    return out


def _build_planes(src_b, tgt_b):
    """Augmented K=32 bf16 planes for one batch.

    Returns L [32, N] (source side / lhsT) and R [32, M] (target side / rhs)
    such that sum_k L[k, n] * R[k, m] == ||s_n - t_m||^2 up to fp32 rounding.
    """
    sa, sb, sc = _splitn(-2.0 * src_b.astype(np.float64), 3)  # (N, 3) each
    ta, tb, tc = _splitn(tgt_b.astype(np.float64), 3)
    ns = (src_b.astype(np.float64) ** 2).sum(1)
    nt = (tgt_b.astype(np.float64) ** 2).sum(1)
    nss = _splitn(ns, 4)
    nts = _splitn(nt, 4)
    one_s = np.ones(ns.shape, BF16)
    one_t = np.ones(nt.shape, BF16)
    Ls, Rs = [], []
    for k in range(3):
        # products: ad ae af bd be bf cd ce (only c*f dropped, ~2^-32 rel)
        for (u, v) in [(sa, ta), (sa, tb), (sa, tc), (sb, ta), (sb, tb), (sb, tc), (sc, ta), (sc, tb)]:
            Ls.append(u[:, k])
            Rs.append(v[:, k])
    for u in nss:
        Ls.append(u)
        Rs.append(one_t)
    for v in nts:
        Ls.append(one_s)
        Rs.append(v)
    L = np.ascontiguousarray(np.stack(Ls, 0).astype(BF16))
    R = np.ascontiguousarray(np.stack(Rs, 0).astype(BF16))
    return L, R


def _build_program():
    """Build the SPMD Tile program once. Returns the finalized Bass object."""
    import concourse.bacc as bacc
    import concourse.tile as tile
    from concourse import mybir

    ttmax = _get_ttmax_op()

    nc = bacc.Bacc("TRN2", target_bir_lowering=False, debug=False, num_devices=NCORES)

    # lhsT packs PAIRS of 128-source blocks at partition strips 0-31 / 32-63
    # so the two K=32 matmuls run concurrently on distinct PE row-groups.
    lhsT_d = nc.dram_tensor("lhsT", [2 * KROWS, SRC_PER_CORE // 2], mybir.dt.bfloat16, kind="ExternalInput")
    rhs_d = nc.dram_tensor("rhs", [2 * KROWS, M], mybir.dt.bfloat16, kind="ExternalInput")
    s2t_d = nc.dram_tensor("s2t", [128, NBLK], mybir.dt.float32, kind="ExternalOutput")
    t2s_d = nc.dram_tensor("t2s", [128, M], mybir.dt.float16, kind="ExternalOutput")

    FMAX = mybir.AluOpType.max
    F16 = mybir.dt.float16

    with tile.TileContext(nc) as tc:
        with (
            tc.tile_pool(name="weights", bufs=1) as wpool,
            tc.tile_pool(name="psum", bufs=2, space="PSUM") as pspool,
            tc.tile_pool(name="evac", bufs=12) as epool,
        ):
            lhsT_sb = wpool.tile([2 * KROWS, SRC_PER_CORE // 2], mybir.dt.bfloat16)
            rhs_sb = wpool.tile([2 * KROWS, M], mybir.dt.bfloat16)
            # Chunked input DMAs: the first matmuls only depend on the first
            # chunks (Tile sub-tile deps), cutting ~12us of startup serial DMA.
            nc.sync.dma_start(out=lhsT_sb[:, 0:256], in_=lhsT_d[:, 0:256])
            for k in range(8):
                nc.sync.dma_start(
                    out=rhs_sb[:, k * 1024:(k + 1) * 1024],
                    in_=rhs_d[:, k * 1024:(k + 1) * 1024],
                )
            nc.sync.dma_start(out=lhsT_sb[:, 256:2048], in_=lhsT_d[:, 256:2048])

            # racc[:, b] = per-source max of -D^2 over ALL targets for block b.
            racc = wpool.tile([128, NBLK], mybir.dt.float32)

            # Per-target accumulator strip over all 4 supertiles.
            A = wpool.tile([128, M], F16, name="A", tag="A")

            NGRP = NBLK // 2
            for g in range(NGRP):
                strips = [
                    epool.tile([128, M], F16, name=f"strip{i}", tag=f"strip{i}", bufs=3)
                    for i in range(2)
                ]
                for c in range(NSUP):
                    ps = [
                        pspool.tile([128, 2048], mybir.dt.float32, name=f"ps{i}", tag=f"ps{i}", bufs=1)
                        for i in range(2)
                    ]
                    for q in range(4):
                        j = c * 4 + q
                        for i in range(2):
                            nc.tensor.matmul(
                                ps[i][:, q * 512:(q + 1) * 512],
                                lhsT_sb[32 * i:32 * (i + 1), g * 128:(g + 1) * 128],
                                rhs_sb[32 * i:32 * (i + 1), j * 512:(j + 1) * 512],
                                start=True,
                                stop=True,
                            )
                    # Evacuate negated (-D^2) so both reductions are max ops.
                    for i in range(2):
                        nc.scalar.activation(
                            strips[i][:, c * 2048:(c + 1) * 2048],
                            ps[i],
                            mybir.ActivationFunctionType.Copy,
                            scale=-1.0,
                        )
                        if g == 0:
                            # Copy-init/fold per evac so DVE starts without
                            # waiting for the second strip (copy runs at 4x).
                            cs0 = slice(c * 2048, (c + 1) * 2048)
                            if i == 0:
                                nc.vector.tensor_copy(A[:, cs0], strips[0][:, cs0])
                            else:
                                nc.vector.tensor_tensor(A[:, cs0], A[:, cs0], strips[1][:, cs0], FMAX)

                    if 0 < g < 4:
                        # Pipeline-fill: fold per supertile so DVE starts
                        # right after each evac pair instead of per strip.
                        cs = slice(c * 2048, (c + 1) * 2048)
                        nc.vector.tensor_tensor(A[:, cs], A[:, cs], strips[0][:, cs], FMAX)
                        nc.vector.tensor_tensor(A[:, cs], A[:, cs], strips[1][:, cs], FMAX)

                # Fold into the per-target accumulator (fp16 TT at 2x, FD=8192).
                if g > 3:
                    nc.vector.tensor_tensor(A, A, strips[0], FMAX)
                    nc.vector.tensor_tensor(A, A, strips[1], FMAX)

                # Forward: ONE fused custom-DVE scan op per strip -- in-place
                # running max of the halves; the last output column holds the
                # full row max, copied into racc.  perf_max=1 arms the 2x_1P
                # table variant (RTL falls back to the 1x scan if declined).
                for i in range(2):
                    b = 2 * g + i
                    s = strips[i]
                    bi = nc.vector._custom_dve(
                        ttmax,
                        out=s[:, 0:4096],
                        in0=s[:, 0:4096],
                        in1=s[:, 4096:8192],
                    )
                    bi.ins.perf_max = 1
                    nc.vector.tensor_copy(racc[:, b:b + 1], s[:, 4095:4096])

            nc.sync.dma_start(out=t2s_d[:, :], in_=A)
            nc.sync.dma_start(out=s2t_d[:, :], in_=racc)

    nc.compile()
    return nc


def _make_in_maps(source, target):
    """Build per-core input dicts (packed lhsT pairs + 2x-replicated rhs)."""
    planes = [_build_planes(source[b], target[b]) for b in range(B)]
    in_maps = []
    for i in range(NCORES):
        b, half = i // 2, i % 2
        L, R = planes[b]
        Lh = L[:, half * SRC_PER_CORE:(half + 1) * SRC_PER_CORE]
        L2 = np.zeros((2 * KROWS, SRC_PER_CORE // 2), BF16)
        for g in range(NBLK // 2):
            for j in range(2):
                L2[32 * j:32 * (j + 1), g * 128:(g + 1) * 128] = \
                    Lh[:, (2 * g + j) * 128:(2 * g + j + 1) * 128]
        R2 = np.concatenate([R, R], axis=0)
        in_maps.append({
            "lhsT": np.ascontiguousarray(L2),
            "rhs": np.ascontiguousarray(R2),
        })
    return in_maps


def _get_program():
    global _PROGRAM
    if _PROGRAM is None:
        _PROGRAM = _build_program()
    return _PROGRAM


def kernel(source, target, weights):
    from concourse.bass_utils import run_bass_kernel_spmd

    source = np.asarray(source)
    target = np.asarray(target)
    weights = np.asarray(weights)

    in_maps = _make_in_maps(source, target)

    nc = _get_program()
    res = None
    last_err = None
    for attempt in range(3):
        try:
            res = run_bass_kernel_spmd(nc, in_maps, list(range(NCORES))).results
            break
        except Exception as e:  # transient device wedge: retry
            last_err = e
            import time as _time

            _time.sleep(5.0 * (attempt + 1))
    if res is None:
        raise last_err

    s_minsq = np.empty((B, N), np.float64)
    t_minsq = np.empty((B, M), np.float64)
    for b in range(B):
        # s2t [128, 32] holds -min D^2: source n = blk*128 + p
        lo = -res[2 * b]["s2t"].astype(np.float64).T.reshape(-1)
        hi = -res[2 * b + 1]["s2t"].astype(np.float64).T.reshape(-1)
        s_minsq[b] = np.maximum(np.concatenate([lo, hi]), 0.0)
        fold = np.maximum(
            res[2 * b]["t2s"].astype(np.float64),
            res[2 * b + 1]["t2s"].astype(np.float64),
        )
        t_minsq[b] = np.maximum(-fold.max(0), 0.0)

    fwd = float((np.sqrt(s_minsq + EPS) * weights.astype(np.float64)).mean())
    bwd = float(np.sqrt(t_minsq + EPS).mean())
    return np.float32(fwd + bwd)
